# revision 1
# baseline (speedup 1.0000x reference)
"""AssignIndex kernel for Trainium2 (8 NeuronCores).

out = arr, except out[index] = element.

Strategy (per sharding hint): shard arr row-wise across the 8 cores
(8192 rows x 1024 f32 = 32 MiB per core). All cores run the identical
SPMD graph: DMA-copy their shard DRAM->DRAM, except local row
`index % rows_per_core-of-owner` which is written from a per-core
"patch" input. For the owner core patch == element; for all other
cores patch == their own original row at that local offset, so the
write is a data no-op and the single SPMD graph stays correct.
"""

import os
import sys

sys.path.insert(0, "/opt/trn_rl_repo")

import numpy as np

N_CORES = 8

# Populated with the most recent BassKernelResults (exec_time_ns etc.)
LAST_RESULT = None


def _build(rows_per_core: int, D: int, local_row: int):
    import concourse.bass as bass
    import concourse.mybir as mybir

    nc = bass.Bass()
    arr = nc.declare_dram_parameter(
        "arr", [rows_per_core, D], mybir.dt.float32, isOutput=False
    )
    patch = nc.declare_dram_parameter(
        "patch", [1, D], mybir.dt.float32, isOutput=False
    )
    out = nc.declare_dram_parameter(
        "out", [rows_per_core, D], mybir.dt.float32, isOutput=True
    )

    with (
        nc.Block() as block,
        nc.semaphore("dma_sem") as dma_sem,
    ):

        @block.sync
        def _(sync):
            expected = 0
            # Three disjoint regions -> no ordering constraints between them.
            if local_row > 0:
                sync.dma_start(
                    out=out[0:local_row], in_=arr[0:local_row]
                ).then_inc(dma_sem, 16)
                expected += 16
            sync.dma_start(
                out=out[local_row : local_row + 1], in_=patch[:]
            ).then_inc(dma_sem, 16)
            expected += 16
            if local_row + 1 < rows_per_core:
                sync.dma_start(
                    out=out[local_row + 1 :], in_=arr[local_row + 1 :]
                ).then_inc(dma_sem, 16)
                expected += 16
            sync.wait_ge(dma_sem, expected)

    return nc


def kernel(arr, index, element):
    global LAST_RESULT
    from concourse.bass_utils import run_bass_kernel_spmd

    arr = np.ascontiguousarray(np.asarray(arr, dtype=np.float32))
    element = np.ascontiguousarray(np.asarray(element, dtype=np.float32))
    N, D = arr.shape
    idx = int(index)
    rows = N // N_CORES
    owner, local = divmod(idx, rows)

    in_maps = []
    for c in range(N_CORES):
        shard = arr[c * rows : (c + 1) * rows]
        p = element if c == owner else shard[local]
        in_maps.append(
            {"arr": shard, "patch": np.ascontiguousarray(p.reshape(1, D))}
        )

    nc = _build(rows, D, local)
    res = run_bass_kernel_spmd(nc, in_maps, core_ids=list(range(N_CORES)))
    LAST_RESULT = res
    return np.concatenate([res.results[c]["out"] for c in range(N_CORES)], axis=0)


# revision 2
# speedup vs baseline: 1.6172x; 1.6172x over previous
"""AssignIndex kernel for Trainium2 (8 NeuronCores).

out = arr, except out[index] = element.

Strategy (per sharding hint): shard arr row-wise across the 8 cores
(8192 rows x 1024 f32 = 32 MiB per core). All cores run the identical
SPMD graph: DMA-copy their shard DRAM->DRAM, except local row
`index % rows_per_core-of-owner` which is written from a per-core
"patch" input. For the owner core patch == element; for all other
cores patch == their own original row at that local offset, so the
write is a data no-op and the single SPMD graph stays correct.
"""

import os
import sys

sys.path.insert(0, "/opt/trn_rl_repo")

import numpy as np

N_CORES = 8

# Populated with the most recent BassKernelResults (exec_time_ns etc.)
LAST_RESULT = None


def _build(rows_per_core: int, D: int, local_row: int):
    import concourse.bass as bass
    import concourse.mybir as mybir

    nc = bass.Bass()
    arr = nc.declare_dram_parameter(
        "arr", [rows_per_core, D], mybir.dt.float32, isOutput=False
    )
    patch = nc.declare_dram_parameter(
        "patch", [1, D], mybir.dt.float32, isOutput=False
    )
    out = nc.declare_dram_parameter(
        "out", [rows_per_core, D], mybir.dt.float32, isOutput=True
    )

    with (
        nc.Block() as block,
        nc.semaphore("dma_sem") as dma_sem,
        nc.semaphore("dma_sem2") as dma_sem2,
    ):
        # Three disjoint regions -> no ordering constraints between them.
        # Region before / after the patched row go on separate HWDGE
        # queues (sync + scalar) so descriptor generation overlaps.

        @block.sync
        def _(sync):
            expected = 0
            if local_row > 0:
                sync.dma_start(
                    out=out[0:local_row], in_=arr[0:local_row]
                ).then_inc(dma_sem, 16)
                expected += 16
            sync.dma_start(
                out=out[local_row : local_row + 1], in_=patch[:]
            ).then_inc(dma_sem, 16)
            expected += 16
            sync.wait_ge(dma_sem, expected)

        @block.scalar
        def _(scalar):
            if local_row + 1 < rows_per_core:
                scalar.dma_start(
                    out=out[local_row + 1 :], in_=arr[local_row + 1 :]
                ).then_inc(dma_sem2, 16)
                scalar.wait_ge(dma_sem2, 16)

    return nc


def kernel(arr, index, element):
    global LAST_RESULT
    from concourse.bass_utils import run_bass_kernel_spmd

    arr = np.ascontiguousarray(np.asarray(arr, dtype=np.float32))
    element = np.ascontiguousarray(np.asarray(element, dtype=np.float32))
    N, D = arr.shape
    idx = int(index)
    rows = N // N_CORES
    owner, local = divmod(idx, rows)

    in_maps = []
    for c in range(N_CORES):
        shard = arr[c * rows : (c + 1) * rows]
        p = element if c == owner else shard[local]
        in_maps.append(
            {"arr": shard, "patch": np.ascontiguousarray(p.reshape(1, D))}
        )

    nc = _build(rows, D, local)
    res = run_bass_kernel_spmd(nc, in_maps, core_ids=list(range(N_CORES)))
    LAST_RESULT = res
    return np.concatenate([res.results[c]["out"] for c in range(N_CORES)], axis=0)
